# revision 8
# baseline (speedup 1.0000x reference)
"""Trainium2 Bass kernel for nn_Classification2 (histogram_binning).

matrix[x, y] = -mean((clip1[y] - clip2[x])**2) * 1e13 over D = 3*224*224
             = -(SCALE/D) * (||a_x||^2 + ||b_y||^2 - 2 a_x.b_y)
output[k]    = mean of matrix over diagonals y - x = k - 64, k in [0, 129)

Strategy: data-parallel over D across 8 NeuronCores. The device computes
ONLY the cross gram a@b^T; the O(S*D) norm vectors and the O(S^2) diagonal
binning are cheap host-side work (0.4% of the FLOPs) done in the
gather/unshard step, alongside the sharding transpose the host already does.

The host pre-transposes each core's D-shard into an fp8-e4m3 [p=128,
f=147, 256] tensor whose columns are [A_f | B_f], so the device DMA is one
contiguous stream and the PE contracts over the partition axis with no
on-chip transposes. Per f-chunk the PE runs one N=128 matmul (lhsT=A_f,
rhs=B_f) accumulating the gram in fp32 PSUM. fp8 halves HBM traffic vs
bf16 (4.8 MB/core) — the kernel is DMA-bound, and the quantization noise
is a ~1e-5 relative error on the final means (validated vs the f32
reference; the budget is 2e-2).

Input DMAs are issued as ramped chunks alternating the two HWDGE issue
engines; the PE consumes f-chunks ~as fast as DMA lands them (81ns/MM vs
~94ns/chunk DMA), so total time ~= DMA stream time + fixed overheads.
"""

import sys

sys.path.insert(0, "/opt/trn_rl_repo")

import numpy as np

S = 128
D = 150528  # 3*224*224
N_CORES = 8
DC = D // N_CORES  # 18816 d-values per core
F = DC // S  # 147 contraction chunks of K=128
# chunk schedule (f units): ALL input chunks on ONE queue (sync/Q1) so bytes
# arrive in the same order the PE consumes them — two parallel queues
# deliver later chunks' bytes early, which delays each chunk's in-order
# completion and stalls the PE. A single InstDMACopy still fans across all
# 16 SDMA engines, so one queue reaches full HBM bandwidth. Small tail
# chunk so the post-stream matmul drain is one MM. Output goes on scalar.
CHUNK_F = [10, 12, 16, 16, 16, 16, 16, 16, 16, 8, 4, 1]
assert sum(CHUNK_F) == F
SCALE = 1.0e13

_NC_CACHE = {}


def _build():
    import concourse.bacc as bacc
    import concourse.mybir as mybir
    import concourse.tile as tile

    f32 = mybir.dt.float32
    bf16 = mybir.dt.bfloat16
    f8 = mybir.dt.float8e4

    nc = bacc.Bacc(num_devices=N_CORES)

    ab_in = nc.dram_tensor("ab", [S, F * 256], f8, kind="ExternalInput")
    out_t = nc.dram_tensor("out", [S * S], bf16, kind="ExternalOutput")

    with tile.TileContext(nc) as tc:
        with (
            tc.tile_pool(name="ab_pool", bufs=1) as ab_pool,
            tc.tile_pool(name="misc", bufs=1) as misc,
            tc.tile_pool(name="psum", bufs=1, space="PSUM") as psum,
        ):
            ab_tiles = []
            f0 = 0
            for ci, nf in enumerate(CHUNK_F):
                t = ab_pool.tile([S, nf * 256], f8, tag=f"ab{ci}")
                sl = slice(f0 * 256, (f0 + nf) * 256)
                nc.sync.dma_start(out=t[:, 0 : nf * 256], in_=ab_in[:, sl])
                ab_tiles.append((t, f0, nf))
                f0 += nf

            ps = psum.tile([S, S], f32, tag="ps")

            for t, f0, nf in ab_tiles:
                for j in range(nf):
                    f = f0 + j
                    base = j * 256
                    nc.tensor.matmul(
                        ps[:, :],
                        t[:, base : base + S],
                        t[:, base + S : base + 256],
                        start=(f == 0),
                        stop=(f == F - 1),
                    )

            g_sb = misc.tile([S, S], bf16, tag="g_sb")
            nc.vector.tensor_copy(g_sb[:, :], ps[:, :])
            nc.scalar.dma_start(
                out=out_t[:].rearrange("(p y) -> p y", p=S),
                in_=g_sb[:, :],
            )

    nc.finalize()
    return nc


def _get_nc():
    if "nc" not in _NC_CACHE:
        _NC_CACHE["nc"] = _build()
    return _NC_CACHE["nc"]


def _shards(clip1: np.ndarray, clip2: np.ndarray):
    """Per-core fp8 [S, F*256] tensors: cols [A_f | B_f] per f, where
    A (p, f, x) = clip2[x, d0 + f*128 + p] (stationary) and
    B (p, f, y) = clip1[y, d0 + f*128 + p] (moving)."""
    import ml_dtypes

    f8 = ml_dtypes.float8_e4m3
    c1 = np.ascontiguousarray(np.asarray(clip1), dtype=np.float32).reshape(S, D)
    c2 = np.ascontiguousarray(np.asarray(clip2), dtype=np.float32).reshape(S, D)
    maps = []
    for c in range(N_CORES):
        sl = slice(c * DC, (c + 1) * DC)
        at = c2[:, sl].reshape(S, F, S).transpose(2, 1, 0)  # [p, f, x] stationary
        bt = c1[:, sl].reshape(S, F, S).transpose(2, 1, 0)  # [p, f, y] moving
        ab = np.empty((S, F, 256), dtype=f8)
        ab[:, :, 0:S] = at.astype(f8)
        ab[:, :, S:256] = bt.astype(f8)
        maps.append({"ab": ab.reshape(S, F * 256)})
    return maps, c1, c2


def _combine(results, c1: np.ndarray, c2: np.ndarray) -> np.ndarray:
    gram = np.zeros((S, S), dtype=np.float64)
    for r in results:
        gram += np.asarray(r["out"], dtype=np.float64).reshape(S, S)
    sq_a = (c2.astype(np.float64) ** 2).sum(axis=1)  # [x] = ||clip2_x||^2
    sq_b = (c1.astype(np.float64) ** 2).sum(axis=1)  # [y] = ||clip1_y||^2
    matrix = -((sq_a[:, None] + sq_b[None, :] - 2.0 * gram) / D) * SCALE
    i = np.arange(S)
    col = (S - 1) - i[:, None] + i[None, :]
    sums = np.zeros(2 * S - 1, dtype=np.float64)
    np.add.at(sums, col, matrix)
    counts = np.concatenate([np.arange(1, S), np.arange(S, 0, -1)]).astype(
        np.float64
    )
    res = sums / counts
    return res[S // 2 - 1 : (S * 3) // 2].astype(np.float32)


def kernel(clip1: np.ndarray, clip2: np.ndarray, **_ignored) -> np.ndarray:
    from concourse.bass_utils import run_bass_kernel_spmd

    in_maps, c1, c2 = _shards(clip1, clip2)
    nc = _get_nc()
    res = run_bass_kernel_spmd(nc, in_maps, core_ids=list(range(N_CORES)))
    return _combine(res.results, c1, c2)


# revision 10
# speedup vs baseline: 1.1641x; 1.1641x over previous
"""Trainium2 Bass kernel for nn_Classification2 (histogram_binning).

matrix[x, y] = -mean((clip1[y] - clip2[x])**2) * 1e13 over D = 3*224*224
             = -(SCALE/D) * (||a_x||^2 + ||b_y||^2 - 2 a_x.b_y)
output[k]    = mean of matrix over diagonals y - x = k - 64, k in [0, 129)

Strategy: data-parallel over D across 8 NeuronCores. The device computes
ONLY the cross gram a@b^T; the O(S*D) norm vectors and the O(S^2) diagonal
binning are cheap host-side work (0.4% of the FLOPs) done in the
gather/unshard step, alongside the sharding transpose the host already does.

The host pre-transposes each core's D-shard into an fp8-e4m3 [p=128,
f=147, 256] tensor whose columns are [A_f | B_f], so the device DMA is one
contiguous stream and the PE contracts over the partition axis with no
on-chip transposes. Per f-chunk the PE runs one N=128 matmul (lhsT=A_f,
rhs=B_f) accumulating the gram in fp32 PSUM. fp8 halves HBM traffic vs
bf16 (4.8 MB/core) — the kernel is DMA-bound, and the quantization noise
is a ~1e-5 relative error on the final means (validated vs the f32
reference; the budget is 2e-2).

Input DMAs are issued as ramped chunks alternating the two HWDGE issue
engines; the PE consumes f-chunks ~as fast as DMA lands them (81ns/MM vs
~94ns/chunk DMA), so total time ~= DMA stream time + fixed overheads.
"""

import sys

sys.path.insert(0, "/opt/trn_rl_repo")

import numpy as np

S = 128
D = 150528  # 3*224*224
N_CORES = 8
DC = D // N_CORES  # 18816 d-values per core
F = DC // S  # 147 contraction chunks of K=128
# chunk schedule (f units, queue): two HWDGE queues so issue overhead
# overlaps (a single queue goes issue-bound at ~0.65us per dma_start and
# starves the ring). Mid chunks kept <=16f so the PE's in-order
# chunk-completion waits stay short, and both queues end with a tiny chunk
# so the post-stream matmul drain is minimal. Output goes on scalar.
CHUNKS = [
    (8, 0), (8, 1), (16, 0), (16, 1), (16, 0), (16, 1),
    (14, 0), (14, 1), (12, 0), (12, 1), (8, 0), (6, 1), (1, 0),
]
assert sum(nf for nf, _ in CHUNKS) == F
SCALE = 1.0e13

_NC_CACHE = {}


def _build():
    import concourse.bacc as bacc
    import concourse.mybir as mybir
    import concourse.tile as tile

    f32 = mybir.dt.float32
    bf16 = mybir.dt.bfloat16
    f8 = mybir.dt.float8e4

    nc = bacc.Bacc(num_devices=N_CORES)

    ab_in = nc.dram_tensor("ab", [S, F * 256], f8, kind="ExternalInput")
    out_t = nc.dram_tensor("out", [S * S], bf16, kind="ExternalOutput")

    with tile.TileContext(nc) as tc:
        with (
            tc.tile_pool(name="ab_pool", bufs=1) as ab_pool,
            tc.tile_pool(name="misc", bufs=1) as misc,
            tc.tile_pool(name="psum", bufs=1, space="PSUM") as psum,
        ):
            ab_tiles = []
            f0 = 0
            for ci, (nf, q) in enumerate(CHUNKS):
                t = ab_pool.tile([S, nf * 256], f8, tag=f"ab{ci}")
                sl = slice(f0 * 256, (f0 + nf) * 256)
                eng = nc.sync if q == 0 else nc.scalar
                eng.dma_start(out=t[:, 0 : nf * 256], in_=ab_in[:, sl])
                ab_tiles.append((t, f0, nf))
                f0 += nf

            ps = psum.tile([S, S], f32, tag="ps")

            for t, f0, nf in ab_tiles:
                for j in range(nf):
                    f = f0 + j
                    base = j * 256
                    nc.tensor.matmul(
                        ps[:, :],
                        t[:, base : base + S],
                        t[:, base + S : base + 256],
                        start=(f == 0),
                        stop=(f == F - 1),
                    )

            g_sb = misc.tile([S, S], bf16, tag="g_sb")
            nc.vector.tensor_copy(g_sb[:, :], ps[:, :])
            nc.scalar.dma_start(
                out=out_t[:].rearrange("(p y) -> p y", p=S),
                in_=g_sb[:, :],
            )

    nc.finalize()
    return nc


def _get_nc():
    if "nc" not in _NC_CACHE:
        _NC_CACHE["nc"] = _build()
    return _NC_CACHE["nc"]


def _shards(clip1: np.ndarray, clip2: np.ndarray):
    """Per-core fp8 [S, F*256] tensors: cols [A_f | B_f] per f, where
    A (p, f, x) = clip2[x, d0 + f*128 + p] (stationary) and
    B (p, f, y) = clip1[y, d0 + f*128 + p] (moving)."""
    import ml_dtypes

    f8 = ml_dtypes.float8_e4m3
    c1 = np.ascontiguousarray(np.asarray(clip1), dtype=np.float32).reshape(S, D)
    c2 = np.ascontiguousarray(np.asarray(clip2), dtype=np.float32).reshape(S, D)
    maps = []
    for c in range(N_CORES):
        sl = slice(c * DC, (c + 1) * DC)
        at = c2[:, sl].reshape(S, F, S).transpose(2, 1, 0)  # [p, f, x] stationary
        bt = c1[:, sl].reshape(S, F, S).transpose(2, 1, 0)  # [p, f, y] moving
        ab = np.empty((S, F, 256), dtype=f8)
        ab[:, :, 0:S] = at.astype(f8)
        ab[:, :, S:256] = bt.astype(f8)
        maps.append({"ab": ab.reshape(S, F * 256)})
    return maps, c1, c2


def _combine(results, c1: np.ndarray, c2: np.ndarray) -> np.ndarray:
    gram = np.zeros((S, S), dtype=np.float64)
    for r in results:
        gram += np.asarray(r["out"], dtype=np.float64).reshape(S, S)
    sq_a = (c2.astype(np.float64) ** 2).sum(axis=1)  # [x] = ||clip2_x||^2
    sq_b = (c1.astype(np.float64) ** 2).sum(axis=1)  # [y] = ||clip1_y||^2
    matrix = -((sq_a[:, None] + sq_b[None, :] - 2.0 * gram) / D) * SCALE
    i = np.arange(S)
    col = (S - 1) - i[:, None] + i[None, :]
    sums = np.zeros(2 * S - 1, dtype=np.float64)
    np.add.at(sums, col, matrix)
    counts = np.concatenate([np.arange(1, S), np.arange(S, 0, -1)]).astype(
        np.float64
    )
    res = sums / counts
    return res[S // 2 - 1 : (S * 3) // 2].astype(np.float32)


def kernel(clip1: np.ndarray, clip2: np.ndarray, **_ignored) -> np.ndarray:
    from concourse.bass_utils import run_bass_kernel_spmd

    in_maps, c1, c2 = _shards(clip1, clip2)
    nc = _get_nc()
    res = run_bass_kernel_spmd(nc, in_maps, core_ids=list(range(N_CORES)))
    return _combine(res.results, c1, c2)
